# revision 33
# baseline (speedup 1.0000x reference)
"""Multi-head causal attention on 8 Trainium2 NeuronCores.

Sharding: tensor-parallel over heads x data-parallel over batch.
Core c handles batch c//4 and heads [4*(c%4), 4*(c%4)+4). Each core
computes Q/K/V projections for its head slice over the full sequence,
causal flash-style attention (transposed scores, ones-column softmax
denominator), and a partial output projection against its row-slice of
W_o. The 4 partial outputs per batch are summed on the host (the
all-reduce of row-parallel W_o), which also adds b_o.

The kernel body is software-pipelined over 8 rounds of 256 queries
each: attention for round r (score matmuls -> exp -> causal-mask mul ->
AV matmuls, with the AV close lagging one unit behind) is interleaved at
score-tile granularity with the Q/K/V projections for the next 512-query
block and the output projection of the previous one. The fine round
granularity spreads the exp() load evenly across the schedule so neither
the tensor engine nor the scalar engine ever drains. Score matmuls use
K=64 tile_position row packing ((0,0)/(64,0)), which streams both heads'
scores concurrently through the PE array; the diagonal 128-key tiles are
computed at 256-query granularity to skip most fully-masked work.
"""
import sys

sys.path.insert(0, '/opt/trn_rl_repo')

import numpy as np
import ml_dtypes

B, S, D, H, DK = 2, 2048, 1024, 16, 64
NCORES = 8
HL = 4            # heads per core
DL = HL * DK      # head-dim slice per core (256)
NQB = S // 512    # 512-wide query blocks
NKST = S // 128   # 128-wide key tiles

_cache = {}


def _build(repeat=1, dynamic=False, stage=4):
    """stage: 1=DMAs only, 2=+QKV projections, 3=+attention, 4=full."""
    import concourse.bacc as bacc
    import concourse.mybir as mybir
    import concourse.tile as tile
    from contextlib import ExitStack, nullcontext

    f32, bf16 = mybir.dt.float32, mybir.dt.bfloat16
    Exp = mybir.ActivationFunctionType.Exp

    nc = bacc.Bacc("TRN2", target_bir_lowering=False, debug=False, num_devices=NCORES)
    xt_d = nc.dram_tensor("xt", (D, S), bf16, kind="ExternalInput").ap()
    wq_d = nc.dram_tensor("wq", (D, DL), bf16, kind="ExternalInput").ap()
    wk_d = nc.dram_tensor("wk", (D, DL), bf16, kind="ExternalInput").ap()
    wv_d = nc.dram_tensor("wv", (D, DL), bf16, kind="ExternalInput").ap()
    wo_d = nc.dram_tensor("wo", (DL, D), bf16, kind="ExternalInput").ap()
    bqk_d = nc.dram_tensor("bqk", (DL, 2), f32, kind="ExternalInput").ap()
    bv_d = nc.dram_tensor("bv", (DL,), f32, kind="ExternalInput").ap()
    mask_d = nc.dram_tensor("masks", (4, 128, 512), bf16, kind="ExternalInput").ap()
    po_d = nc.dram_tensor("po", (128, NQB, 8, 512), bf16, kind="ExternalOutput").ap()

    with tile.TileContext(nc) as tc:
        with ExitStack() as ctx:
            sb = ctx.enter_context(tc.tile_pool(name="sb", bufs=1))
            ps = ctx.enter_context(tc.tile_pool(name="ps", bufs=1, space="PSUM"))

            # ---- persistent SBUF tiles (per 512-block for precise deps) ----
            xt_t = [sb.tile([128, 8, 512], bf16, name=f"xt{b_}") for b_ in range(NQB)]
            wq_s = sb.tile([128, 8, DL], bf16, name="wq_s")
            wk_s = sb.tile([128, 8, DL], bf16, name="wk_s")
            wv_s = sb.tile([128, 8, DL], bf16, name="wv_s")
            wo_s = sb.tile([128, 2, D], bf16, name="wo_s")
            qt = [[sb.tile([128, 512], bf16, name=f"qt{p}_{b_}") for b_ in range(NQB)]
                  for p in range(2)]
            kt = [[sb.tile([128, 512], bf16, name=f"kt{p}_{b_}") for b_ in range(NQB)]
                  for p in range(2)]
            ctxt = [[sb.tile([128, 512], bf16, name=f"ct{p}_{b_}") for b_ in range(NQB)]
                    for p in range(2)]
            # per key-tile V, augmented: 2 pairs x (64 even | one | 64 odd | spare)
            va = [sb.tile([128, 264], bf16, name=f"va{sp}") for sp in range(NKST)]
            masks = sb.tile([128, 4, 512], bf16, name="masks")
            bqk_t = sb.tile([128, 2, 2], f32, name="bqk_t")
            bv_sb = sb.tile([1, DL], f32, name="bv_sb")
            bvB = sb.tile([128, DL], f32, name="bvB")

            rep_ctx = tc.For_i(0, repeat, 1) if dynamic else nullcontext(range(repeat))
            with rep_ctx as _it:
              for _rep in ([0] if dynamic else _it):
                # ---- input DMAs: sync ring: xt by block; scalar: weights ----
                # sync ring carries xt + wk only (wk halves interleaved with
                # the xt[0] halves the first K-proj needs); everything else
                # rides SWDGE on the mostly-idle gpsimd queue so the ACT
                # (scalar) sequencer stays exp-only
                xt_r = xt_d.rearrange("(k p) s -> p k s", p=128)
                wk_r = wk_d.rearrange("(k p) n -> p k n", p=128)
                nc.sync.dma_start(xt_t[0][:, 0:4, :], xt_r[:, 0:4, 0:512])
                nc.sync.dma_start(wk_s[:, 0:4, :], wk_r[:, 0:4, :])
                nc.sync.dma_start(xt_t[0][:, 4:8, :], xt_r[:, 4:8, 0:512])
                nc.sync.dma_start(wk_s[:, 4:8, :], wk_r[:, 4:8, :])
                for b_ in range(1, NQB):
                    nc.sync.dma_start(xt_t[b_][:], xt_r[:, :, b_ * 512:(b_ + 1) * 512])
                nc.gpsimd.dma_start(wq_s[:], wq_d.rearrange("(k p) n -> p k n", p=128))
                nc.gpsimd.dma_start(bqk_t[:], bqk_d.rearrange("(p2 p) j -> p p2 j", p=128))
                nc.gpsimd.dma_start(wv_s[:], wv_d.rearrange("(k p) n -> p k n", p=128))
                nc.gpsimd.dma_start(wo_s[:], wo_d.rearrange("(k p) n -> p k n", p=128))
                nc.gpsimd.dma_start(masks[:], mask_d.rearrange("t p n -> p t n"))
                nc.gpsimd.dma_start(bv_sb[:], bv_d.rearrange("(o n) -> o n", o=1))
                nc.gpsimd.partition_broadcast(bvB[:], bv_sb[:])

                # ---- unit emitters ----
                def kq_unit(dst, w_s, bcol, p, qb):
                    def u():
                        pp = ps.tile([128, 512], f32, tag="pj", bufs=2)
                        for k in range(8):
                            nc.tensor.matmul(pp[:], w_s[:, k, p * 128:(p + 1) * 128],
                                             xt_t[qb][:, k, :],
                                             start=(k == 0), stop=(k == 7))
                        nc.vector.tensor_scalar_add(dst[p][qb][:], pp[:],
                                                    bqk_t[:, p, bcol:bcol + 1])
                    return u

                def v_unit(sp):
                    def u():
                        pv = ps.tile([128, 512], f32, tag="pj", bufs=2)
                        for k in range(8):
                            nc.tensor.matmul(pv[:, 0:DL],
                                             xt_t[sp // 4][:, k, (sp % 4) * 128:(sp % 4) * 128 + 128],
                                             wv_s[:, k, :], start=(k == 0), stop=(k == 7))
                        vdst = va[sp].rearrange("p (pr e q) -> p pr e q", pr=2, e=2, q=66)
                        nc.vector.tensor_add(
                            vdst[:, :, :, 0:64],
                            pv[:, 0:DL].rearrange("p (pr e q) -> p pr e q", pr=2, e=2, q=64),
                            bvB[:].rearrange("p (pr e q) -> p pr e q", pr=2, e=2, q=64))
                        nc.gpsimd.memset(vdst[:, :, :, 64:65], 1.0)
                    return u

                def op_unit(qb, ot, po_sb, slot, last=False):
                    def u():
                        if last and ot % 2 == 1:
                            # attention is drained in the epilogue: borrow the
                            # idle sc ring so consecutive op units don't wait
                            # on the 2-deep pj ring's psum->sbuf copies
                            po_p = ps.tile([128, 1024], f32, tag="sc", bufs=2)
                        else:
                            po_p = ps.tile([128, 512], f32, tag="pj", bufs=2)
                        for k in range(2):
                            nc.tensor.matmul(po_p[:, 0:512],
                                             wo_s[:, k, ot * 128:(ot + 1) * 128],
                                             ctxt[k][qb][:], start=(k == 0), stop=(k == 1))
                        nc.vector.tensor_copy(po_sb[:, slot, :], po_p[:, 0:512])
                    return u

                def po_dma_unit(qb, po_sb, half):
                    def u():
                        # SWDGE queue: keeps the sync ring free so the next
                        # iteration's xt loads aren't queued behind outputs
                        nc.gpsimd.dma_start(po_d[:, qb, half * 4:half * 4 + 4, :],
                                            po_sb[:])
                    return u

                def _kt_ap(p, kst, rows):
                    blk, off = kst // 4, (kst % 4) * 128
                    return kt[p][blk][rows, off:off + 128]

                def _sc_mm(sc0, sc1, p, qb, kst, dst_sl, q_sl):
                    nc.tensor.matmul(sc0[:, dst_sl], _kt_ap(p, kst, slice(0, 64)),
                                     qt[p][qb][0:64, q_sl],
                                     start=True, stop=True, tile_position=(0, 0))
                    nc.tensor.matmul(sc1[:, dst_sl], _kt_ap(p, kst, slice(64, 128)),
                                     qt[p][qb][64:128, q_sl],
                                     start=True, stop=True, tile_position=(64, 0))

                def _av_mm(av0, av1, kst, e0, e1, e_sl, out_sl, p, st, sp_):
                    nc.tensor.matmul(av0[:, out_sl], va[kst][:, p * 132:p * 132 + 65],
                                     e0[:, e_sl], start=st, stop=sp_)
                    nc.tensor.matmul(av1[:, out_sl], va[kst][:, p * 132 + 66:p * 132 + 131],
                                     e1[:, e_sl], start=st, stop=sp_)

                def attn_unit_open(p, rr, j, av0, av1, first, last):
                    # round rr covers queries [rr*256, rr*256+256); this unit
                    # covers kst 4j .. min(4j+4, 2rr+2)-1. The last two kst of
                    # the round (2rr, 2rr+1) straddle the causal diagonal:
                    # kst 2rr needs masking on its first 128 query columns, and
                    # kst 2rr+1 is FULLY masked for the first 128 queries, so
                    # it is computed only for queries [128:256) of the round
                    # (with the same triangular mask, shifted by 128).
                    qb, qh = rr // 2, rr % 2
                    n_kst = 2 * rr + 2
                    ks = list(range(4 * j, min(4 * j + 4, n_kst)))
                    nk = len(ks)
                    q0 = qh * 256
                    sc0 = ps.tile([128, 1024], f32, tag="sc", bufs=2)
                    sc1 = ps.tile([128, 1024], f32, tag="sc", bufs=2)
                    for i, kst in enumerate(ks):
                        if kst == n_kst - 1:
                            _sc_mm(sc0, sc1, p, qb, kst,
                                   slice(i * 256, i * 256 + 128),
                                   slice(q0 + 128, q0 + 256))
                        else:
                            _sc_mm(sc0, sc1, p, qb, kst,
                                   slice(i * 256, (i + 1) * 256),
                                   slice(q0, q0 + 256))
                    e0 = sb.tile([128, 1024], bf16, tag="ex", bufs=6)
                    e1 = sb.tile([128, 1024], bf16, tag="ex", bufs=6)
                    w = nk * 256 - (128 if ks[-1] == n_kst - 1 else 0)
                    nc.scalar.activation(e0[:, 0:w], sc0[:, 0:w], Exp, scale=0.125)
                    nc.scalar.activation(e1[:, 0:w], sc1[:, 0:w], Exp, scale=0.125)
                    for i, kst in enumerate(ks):
                        if kst in (n_kst - 2, n_kst - 1):
                            for e in (e0, e1):
                                nc.vector.tensor_mul(e[:, i * 256:i * 256 + 128],
                                                     e[:, i * 256:i * 256 + 128],
                                                     masks[:, 0, 0:128])

                    def close():
                        for i, kst in enumerate(ks):
                            if kst == n_kst - 1:
                                _av_mm(av0, av1, kst, e0, e1,
                                       slice(i * 256, i * 256 + 128),
                                       slice(128, 256),
                                       p, st=(first and i == 0),
                                       sp_=(last and i == nk - 1))
                            else:
                                _av_mm(av0, av1, kst, e0, e1,
                                       slice(i * 256, (i + 1) * 256), slice(0, 256),
                                       p, st=(first and i == 0),
                                       sp_=(last and i == nk - 1))
                    return close

                def norm_unit(p, rr, av0, av1):
                    qb, qh = rr // 2, rr % 2
                    q_sl = slice(qh * 256, qh * 256 + 256)
                    rcs, rbs = [], []
                    for e, av in ((0, av0), (1, av1)):
                        rc = sb.tile([1, 256], f32, tag="rc", bufs=4)
                        rb = sb.tile([64, 256], f32, tag="rb", bufs=4)
                        nc.vector.reciprocal(rc[:], av[64:65, 0:256])
                        rcs.append(rc)
                        rbs.append(rb)
                    for e in range(2):
                        nc.gpsimd.partition_broadcast(rbs[e][:], rcs[e][:])
                    for e, av in ((0, av0), (1, av1)):
                        nc.vector.tensor_mul(ctxt[p][qb][e * 64:(e + 1) * 64, q_sl],
                                             av[0:64, 0:256], rbs[e][:])

                def proj_units(qb):
                    return [kq_unit(kt, wk_s, 1, 0, qb), kq_unit(kt, wk_s, 1, 1, qb),
                            kq_unit(qt, wq_s, 0, 0, qb), kq_unit(qt, wq_s, 0, 1, qb),
                            v_unit(4 * qb + 0), v_unit(4 * qb + 1),
                            v_unit(4 * qb + 2), v_unit(4 * qb + 3)]

                def outproj_half(qb, half, last=False):
                    po_sb = sb.tile([128, 4, 512], bf16, tag="po_s", bufs=2)
                    us = [op_unit(qb, half * 4 + i, po_sb, i, last=last)
                          for i in range(4)]
                    us.append(po_dma_unit(qb, po_sb, half))
                    return us

                # ---- pipelined schedule ----
                if stage >= 2:
                    for u in proj_units(0):
                        u()

                pend = None  # (close_fn, norm_fn_or_None)

                def flush_pend():
                    nonlocal pend
                    if pend is not None:
                        close_fn, norm_fn = pend
                        close_fn()
                        if norm_fn is not None:
                            norm_fn()
                        pend = None

                # per-round fillers: K/Q proj of block b lands in round 2b-2,
                # V proj of block b in round 2b-1, outproj of block b split
                # across rounds 2b+2 / 2b+3
                def round_fillers(rr):
                    fs = []
                    if stage >= 2:
                        b = rr // 2 + 1
                        if b < NQB:
                            fs += proj_units(b)[0:4] if rr % 2 == 0 else proj_units(b)[4:8]
                    if stage >= 4 and rr >= 2:
                        fs += outproj_half(rr // 2 - 1, rr % 2)
                    return fs
                for rr in range(2 * NQB if stage >= 3 else 0):
                    fillers = round_fillers(rr)
                    n_u_round = 2 * ((2 * rr + 2 + 3) // 4)
                    gi, fi = 0, 0
                    for p in (0, 1):
                        av0 = ps.tile([65, 256], f32, tag="av", bufs=2)
                        av1 = ps.tile([65, 256], f32, tag="av", bufs=2)
                        n_units = (2 * rr + 2 + 3) // 4
                        for ui in range(n_units):
                            first, last = ui == 0, ui == n_units - 1
                            close = attn_unit_open(p, rr, ui, av0, av1, first, last)
                            tgt = (gi + 1) * len(fillers) // n_u_round
                            while fi < tgt:
                                fillers[fi]()
                                fi += 1
                            flush_pend()
                            nf = ((lambda p_=p, r_=rr, a0=av0, a1=av1:
                                   norm_unit(p_, r_, a0, a1)) if last else None)
                            pend = (close, nf)
                            gi += 1
                flush_pend()

                if stage >= 3 and stage < 4:
                    pass
                if stage == 2:
                    # emit remaining projections (no attention to hide them in)
                    for qb in range(1, NQB):
                        for u in proj_units(qb):
                            u()
                if stage >= 4:
                    for half in range(2):
                        for u in outproj_half(NQB - 1, half, last=True):
                            u()

    nc.compile()
    return nc


def _causal_mask_ok(mask):
    m = np.asarray(mask)
    if m.shape != (S, S):
        return False
    return np.array_equal(m.astype(bool), np.triu(np.ones((S, S), bool), k=1))


def _numpy_fallback(x, mask, Wq, bq, Wk, bk, Wv, bv, Wo, bo):
    x = np.asarray(x, np.float64)
    q = (x @ Wq + bq).reshape(B, S, H, DK).transpose(0, 2, 1, 3)
    k = (x @ Wk + bk).reshape(B, S, H, DK).transpose(0, 2, 1, 3)
    v = (x @ Wv + bv).reshape(B, S, H, DK).transpose(0, 2, 1, 3)
    s = np.einsum("bhqd,bhkd->bhqk", q, k) / np.sqrt(DK)
    s = np.where(np.asarray(mask, bool), -np.inf, s)
    s = s - s.max(-1, keepdims=True)
    e = np.exp(s)
    a = e / e.sum(-1, keepdims=True)
    ctx = np.einsum("bhqk,bhkd->bhqd", a, v).transpose(0, 2, 1, 3).reshape(B, S, D)
    return (ctx @ Wo + bo).astype(np.float32)


def _tri_masks():
    m = np.zeros((4, 128, 512), np.float32)
    n = np.arange(512)
    for t in range(4):
        for p_ in range(128):
            m[t, p_, :] = (n >= t * 128 + p_)
    return m.astype(ml_dtypes.bfloat16)


def _make_in_maps(x, Wq, bq, Wk, bk, Wv, bv, Wo):
    Wq, Wk, Wv, Wo = (np.asarray(w, np.float32) for w in (Wq, Wk, Wv, Wo))
    bq, bk, bv = (np.asarray(b_, np.float32) for b_ in (bq, bk, bv))
    masks_np = _tri_masks()
    xts = [np.ascontiguousarray(x[b_].T.astype(ml_dtypes.bfloat16)) for b_ in range(B)]

    in_maps = []
    for c in range(NCORES):
        b_, hs = c // 4, (c % 4) * DL
        in_maps.append({
            "xt": xts[b_],
            "wq": np.ascontiguousarray(Wq[:, hs:hs + DL].astype(ml_dtypes.bfloat16)),
            "wk": np.ascontiguousarray(Wk[:, hs:hs + DL].astype(ml_dtypes.bfloat16)),
            "wv": np.ascontiguousarray(Wv[:, hs:hs + DL].astype(ml_dtypes.bfloat16)),
            "wo": np.ascontiguousarray(Wo[hs:hs + DL, :].astype(ml_dtypes.bfloat16)),
            "bqk": np.ascontiguousarray(np.stack([bq[hs:hs + DL], bk[hs:hs + DL]], 1)),
            "bv": np.ascontiguousarray(bv[hs:hs + DL]),
            "masks": masks_np,
        })
    return in_maps


def kernel(x, mask, Wq, bq, Wk, bk, Wv, bv, Wo, bo):
    x = np.ascontiguousarray(np.asarray(x, np.float32))
    if not _causal_mask_ok(mask):
        return _numpy_fallback(x, mask, Wq, bq, Wk, bk, Wv, bv, Wo, bo)

    from concourse import bass_utils

    if "nc" not in _cache:
        _cache["nc"] = _build(repeat=1)
    nc = _cache["nc"]

    bo = np.asarray(bo, np.float32)
    in_maps = _make_in_maps(x, Wq, bq, Wk, bk, Wv, bv, Wo)

    res = bass_utils.run_bass_kernel_spmd(nc, in_maps, core_ids=list(range(NCORES)))

    out = np.empty((B, S, D), np.float32)
    for b_ in range(B):
        acc = res.results[b_ * 4]["po"].astype(np.float32)
        for g in range(1, 4):
            acc = acc + res.results[b_ * 4 + g]["po"]
        # acc[p, qb, k, s] = outT[k*128+p, qb*512+s]
        out[b_] = acc.transpose(1, 3, 2, 0).reshape(S, D) + bo
    return out


# revision 35
# speedup vs baseline: 1.0715x; 1.0715x over previous
"""Multi-head causal attention on 8 Trainium2 NeuronCores.

Sharding: tensor-parallel over heads x data-parallel over batch.
Core c handles batch c//4 and heads [4*(c%4), 4*(c%4)+4). Each core
computes Q/K/V projections for its head slice over the full sequence,
causal flash-style attention (transposed scores, ones-column softmax
denominator), and a partial output projection against its row-slice of
W_o. The 4 partial outputs per batch are summed on the host (the
all-reduce of row-parallel W_o), which also adds b_o.

The kernel body is software-pipelined over 8 rounds of 256 queries
each: attention for round r (score matmuls -> exp -> causal-mask mul ->
AV matmuls, with the AV close lagging one unit behind) is interleaved at
score-tile granularity with the Q/K/V projections for the next 512-query
block and the output projection of the previous one. The fine round
granularity spreads the exp() load evenly across the schedule so neither
the tensor engine nor the scalar engine ever drains. Score matmuls use
K=64 tile_position row packing ((0,0)/(64,0)), which streams both heads'
scores concurrently through the PE array; the diagonal 128-key tiles are
computed at 256-query granularity to skip most fully-masked work.
"""
import sys

sys.path.insert(0, '/opt/trn_rl_repo')

import numpy as np
import ml_dtypes

B, S, D, H, DK = 2, 2048, 1024, 16, 64
NCORES = 8
HL = 4            # heads per core
DL = HL * DK      # head-dim slice per core (256)
NQB = S // 512    # 512-wide query blocks
NKST = S // 128   # 128-wide key tiles

_cache = {}


def _build(repeat=1, dynamic=False, stage=4):
    """stage: 1=DMAs only, 2=+QKV projections, 3=+attention, 4=full."""
    import concourse.bacc as bacc
    import concourse.mybir as mybir
    import concourse.tile as tile
    from contextlib import ExitStack, nullcontext

    f32, bf16 = mybir.dt.float32, mybir.dt.bfloat16
    Exp = mybir.ActivationFunctionType.Exp

    nc = bacc.Bacc("TRN2", target_bir_lowering=False, debug=False, num_devices=NCORES)
    xt_d = nc.dram_tensor("xt", (D, S), bf16, kind="ExternalInput").ap()
    wq_d = nc.dram_tensor("wq", (D, DL), bf16, kind="ExternalInput").ap()
    wk_d = nc.dram_tensor("wk", (D, DL), bf16, kind="ExternalInput").ap()
    wv_d = nc.dram_tensor("wv", (D, DL), bf16, kind="ExternalInput").ap()
    wo_d = nc.dram_tensor("wo", (DL, D), bf16, kind="ExternalInput").ap()
    bqk_d = nc.dram_tensor("bqk", (DL, 2), f32, kind="ExternalInput").ap()
    bv_d = nc.dram_tensor("bv", (DL,), f32, kind="ExternalInput").ap()
    mask_d = nc.dram_tensor("masks", (4, 128, 512), bf16, kind="ExternalInput").ap()
    po_d = nc.dram_tensor("po", (128, NQB, 8, 512), bf16, kind="ExternalOutput").ap()

    with tile.TileContext(nc) as tc:
        with ExitStack() as ctx:
            sb = ctx.enter_context(tc.tile_pool(name="sb", bufs=1))
            ps = ctx.enter_context(tc.tile_pool(name="ps", bufs=1, space="PSUM"))

            # ---- persistent SBUF tiles (per 512-block for precise deps) ----
            xt_t = [sb.tile([128, 8, 512], bf16, name=f"xt{b_}") for b_ in range(NQB)]
            wq_s = sb.tile([128, 8, DL], bf16, name="wq_s")
            wk_s = sb.tile([128, 8, DL], bf16, name="wk_s")
            wv_s = sb.tile([128, 8, DL], bf16, name="wv_s")
            wo_s = sb.tile([128, 2, D], bf16, name="wo_s")
            qt = [[sb.tile([128, 512], bf16, name=f"qt{p}_{b_}") for b_ in range(NQB)]
                  for p in range(2)]
            kt = [[sb.tile([128, 512], bf16, name=f"kt{p}_{b_}") for b_ in range(NQB)]
                  for p in range(2)]
            ctxt = [[sb.tile([128, 512], bf16, name=f"ct{p}_{b_}") for b_ in range(NQB)]
                    for p in range(2)]
            # per key-tile V, augmented: 2 pairs x (64 even | one | 64 odd | spare)
            va = [sb.tile([128, 264], bf16, name=f"va{sp}") for sp in range(NKST)]
            masks = sb.tile([128, 4, 512], bf16, name="masks")
            bqk_t = sb.tile([128, 2, 2], f32, name="bqk_t")
            bv_sb = sb.tile([1, DL], f32, name="bv_sb")
            bvB = sb.tile([128, DL], f32, name="bvB")

            rep_ctx = tc.For_i(0, repeat, 1) if dynamic else nullcontext(range(repeat))
            with rep_ctx as _it:
              for _rep in ([0] if dynamic else _it):
                # ---- input DMAs: sync ring: xt by block; scalar: weights ----
                # sync ring carries xt + wk only (wk halves interleaved with
                # the xt[0] halves the first K-proj needs); everything else
                # rides SWDGE on the mostly-idle gpsimd queue so the ACT
                # (scalar) sequencer stays exp-only
                xt_r = xt_d.rearrange("(k p) s -> p k s", p=128)
                wk_r = wk_d.rearrange("(k p) n -> p k n", p=128)
                nc.sync.dma_start(xt_t[0][:, 0:4, :], xt_r[:, 0:4, 0:512])
                nc.sync.dma_start(wk_s[:, 0:4, :], wk_r[:, 0:4, :])
                nc.sync.dma_start(xt_t[0][:, 4:8, :], xt_r[:, 4:8, 0:512])
                nc.sync.dma_start(wk_s[:, 4:8, :], wk_r[:, 4:8, :])
                for b_ in range(1, NQB):
                    nc.sync.dma_start(xt_t[b_][:], xt_r[:, :, b_ * 512:(b_ + 1) * 512])
                nc.gpsimd.dma_start(wq_s[:], wq_d.rearrange("(k p) n -> p k n", p=128))
                nc.gpsimd.dma_start(bqk_t[:], bqk_d.rearrange("(p2 p) j -> p p2 j", p=128))
                nc.gpsimd.dma_start(wv_s[:], wv_d.rearrange("(k p) n -> p k n", p=128))
                nc.gpsimd.dma_start(wo_s[:], wo_d.rearrange("(k p) n -> p k n", p=128))
                nc.gpsimd.dma_start(masks[:], mask_d.rearrange("t p n -> p t n"))
                nc.gpsimd.dma_start(bv_sb[:], bv_d.rearrange("(o n) -> o n", o=1))
                nc.gpsimd.partition_broadcast(bvB[:], bv_sb[:])

                # ---- unit emitters ----
                def kq_unit(dst, w_s, bcol, p, qb):
                    def u():
                        pp = ps.tile([128, 512], f32, tag="pj", bufs=2)
                        for k in range(8):
                            nc.tensor.matmul(pp[:], w_s[:, k, p * 128:(p + 1) * 128],
                                             xt_t[qb][:, k, :],
                                             start=(k == 0), stop=(k == 7))
                        nc.vector.tensor_scalar_add(dst[p][qb][:], pp[:],
                                                    bqk_t[:, p, bcol:bcol + 1])
                    return u

                def v_unit(sp):
                    def u():
                        pv = ps.tile([128, 512], f32, tag="pj", bufs=2)
                        for k in range(8):
                            nc.tensor.matmul(pv[:, 0:DL],
                                             xt_t[sp // 4][:, k, (sp % 4) * 128:(sp % 4) * 128 + 128],
                                             wv_s[:, k, :], start=(k == 0), stop=(k == 7))
                        vdst = va[sp].rearrange("p (pr e q) -> p pr e q", pr=2, e=2, q=66)
                        nc.vector.tensor_add(
                            vdst[:, :, :, 0:64],
                            pv[:, 0:DL].rearrange("p (pr e q) -> p pr e q", pr=2, e=2, q=64),
                            bvB[:].rearrange("p (pr e q) -> p pr e q", pr=2, e=2, q=64))
                        nc.gpsimd.memset(vdst[:, :, :, 64:65], 1.0)
                    return u

                def op_unit(qb, ot, po_sb, slot, last=False):
                    def u():
                        if last and ot % 2 == 1:
                            # attention is drained in the epilogue: borrow the
                            # idle sc ring so consecutive op units don't wait
                            # on the 2-deep pj ring's psum->sbuf copies
                            po_p = ps.tile([128, 1024], f32, tag="sc", bufs=2)
                        else:
                            po_p = ps.tile([128, 512], f32, tag="pj", bufs=2)
                        for k in range(2):
                            nc.tensor.matmul(po_p[:, 0:512],
                                             wo_s[:, k, ot * 128:(ot + 1) * 128],
                                             ctxt[k][qb][:], start=(k == 0), stop=(k == 1))
                        nc.vector.tensor_copy(po_sb[:, slot, :], po_p[:, 0:512])
                    return u

                def po_dma_unit(qb, po_sb, half):
                    def u():
                        # SWDGE queue: keeps the sync ring free so the next
                        # iteration's xt loads aren't queued behind outputs
                        nc.gpsimd.dma_start(po_d[:, qb, half * 4:half * 4 + 4, :],
                                            po_sb[:])
                    return u

                def _kt_ap(p, kst, rows):
                    blk, off = kst // 4, (kst % 4) * 128
                    return kt[p][blk][rows, off:off + 128]

                def _sc_mm(sc0, sc1, p, qb, kst, dst_sl, q_sl):
                    nc.tensor.matmul(sc0[:, dst_sl], _kt_ap(p, kst, slice(0, 64)),
                                     qt[p][qb][0:64, q_sl],
                                     start=True, stop=True, tile_position=(0, 0))
                    nc.tensor.matmul(sc1[:, dst_sl], _kt_ap(p, kst, slice(64, 128)),
                                     qt[p][qb][64:128, q_sl],
                                     start=True, stop=True, tile_position=(64, 0))

                def _av_mm(av0, av1, kst, e0, e1, e_sl, out_sl, p, st, sp_):
                    nc.tensor.matmul(av0[:, out_sl], va[kst][:, p * 132:p * 132 + 65],
                                     e0[:, e_sl], start=st, stop=sp_)
                    nc.tensor.matmul(av1[:, out_sl], va[kst][:, p * 132 + 66:p * 132 + 131],
                                     e1[:, e_sl], start=st, stop=sp_)

                def attn_unit_open(p, rr, j, av0, av1, first, last):
                    # round rr covers queries [rr*256, rr*256+256); this unit
                    # covers kst 4j .. min(4j+4, 2rr+2)-1. The last two kst of
                    # the round (2rr, 2rr+1) straddle the causal diagonal:
                    # kst 2rr needs masking on its first 128 query columns, and
                    # kst 2rr+1 is FULLY masked for the first 128 queries, so
                    # it is computed only for queries [128:256) of the round
                    # (with the same triangular mask, shifted by 128).
                    qb, qh = rr // 2, rr % 2
                    n_kst = 2 * rr + 2
                    ks = list(range(4 * j, min(4 * j + 4, n_kst)))
                    nk = len(ks)
                    q0 = qh * 256
                    sc0 = ps.tile([128, 1024], f32, tag="sc", bufs=2)
                    sc1 = ps.tile([128, 1024], f32, tag="sc", bufs=2)
                    for i, kst in enumerate(ks):
                        if kst == n_kst - 1:
                            _sc_mm(sc0, sc1, p, qb, kst,
                                   slice(i * 256, i * 256 + 128),
                                   slice(q0 + 128, q0 + 256))
                        else:
                            _sc_mm(sc0, sc1, p, qb, kst,
                                   slice(i * 256, (i + 1) * 256),
                                   slice(q0, q0 + 256))
                    e0 = sb.tile([128, 1024], bf16, tag="ex", bufs=6)
                    e1 = sb.tile([128, 1024], bf16, tag="ex", bufs=6)
                    w = nk * 256 - (128 if ks[-1] == n_kst - 1 else 0)
                    nc.scalar.activation(e0[:, 0:w], sc0[:, 0:w], Exp, scale=0.125)
                    nc.scalar.activation(e1[:, 0:w], sc1[:, 0:w], Exp, scale=0.125)
                    for i, kst in enumerate(ks):
                        if kst in (n_kst - 2, n_kst - 1):
                            for e in (e0, e1):
                                nc.vector.tensor_mul(e[:, i * 256:i * 256 + 128],
                                                     e[:, i * 256:i * 256 + 128],
                                                     masks[:, 0, 0:128])

                    def close():
                        for i, kst in enumerate(ks):
                            if kst == n_kst - 1:
                                _av_mm(av0, av1, kst, e0, e1,
                                       slice(i * 256, i * 256 + 128),
                                       slice(128, 256),
                                       p, st=(first and i == 0),
                                       sp_=(last and i == nk - 1))
                            else:
                                _av_mm(av0, av1, kst, e0, e1,
                                       slice(i * 256, (i + 1) * 256), slice(0, 256),
                                       p, st=(first and i == 0),
                                       sp_=(last and i == nk - 1))
                    return close

                def norm_unit(p, rr, av0, av1):
                    qb, qh = rr // 2, rr % 2
                    q_sl = slice(qh * 256, qh * 256 + 256)
                    rcs, rbs = [], []
                    for e, av in ((0, av0), (1, av1)):
                        rc = sb.tile([1, 256], f32, tag="rc", bufs=4)
                        rb = sb.tile([64, 256], f32, tag="rb", bufs=4)
                        nc.vector.reciprocal(rc[:], av[64:65, 0:256])
                        rcs.append(rc)
                        rbs.append(rb)
                    for e in range(2):
                        nc.gpsimd.partition_broadcast(rbs[e][:], rcs[e][:])
                    for e, av in ((0, av0), (1, av1)):
                        nc.vector.tensor_mul(ctxt[p][qb][e * 64:(e + 1) * 64, q_sl],
                                             av[0:64, 0:256], rbs[e][:])

                def proj_units(qb):
                    return [kq_unit(kt, wk_s, 1, 0, qb), kq_unit(kt, wk_s, 1, 1, qb),
                            kq_unit(qt, wq_s, 0, 0, qb), kq_unit(qt, wq_s, 0, 1, qb),
                            v_unit(4 * qb + 0), v_unit(4 * qb + 1),
                            v_unit(4 * qb + 2), v_unit(4 * qb + 3)]

                def outproj_half(qb, half, last=False):
                    po_sb = sb.tile([128, 4, 512], bf16, tag="po_s", bufs=2)
                    us = [op_unit(qb, half * 4 + i, po_sb, i, last=last)
                          for i in range(4)]
                    us.append(po_dma_unit(qb, po_sb, half))
                    return us

                # ---- pipelined schedule ----
                if stage >= 2:
                    for u in proj_units(0):
                        u()

                pend = None  # (close_fn, norm_fn_or_None)

                def flush_pend():
                    nonlocal pend
                    if pend is not None:
                        close_fn, norm_fn = pend
                        close_fn()
                        if norm_fn is not None:
                            norm_fn()
                        pend = None

                # per-round fillers: K/Q proj of block b lands in round 2b-2,
                # V proj of block b in round 2b-1, outproj of block b split
                # across rounds 2b+2 / 2b+3
                def round_fillers(rr):
                    fs = []
                    if stage >= 2:
                        b = rr // 2 + 1
                        if b < NQB:
                            fs += proj_units(b)[0:4] if rr % 2 == 0 else proj_units(b)[4:8]
                    if stage >= 4 and rr >= 2:
                        fs += outproj_half(rr // 2 - 1, rr % 2)
                    return fs
                for rr in range(2 * NQB if stage >= 3 else 0):
                    fillers = round_fillers(rr)
                    n_u_round = 2 * ((2 * rr + 2 + 3) // 4)
                    gi, fi = 0, 0
                    for p in (0, 1):
                        av0 = ps.tile([65, 256], f32, tag="av", bufs=2)
                        av1 = ps.tile([65, 256], f32, tag="av", bufs=2)
                        n_units = (2 * rr + 2 + 3) // 4
                        for ui in range(n_units):
                            first, last = ui == 0, ui == n_units - 1
                            close = attn_unit_open(p, rr, ui, av0, av1, first, last)
                            tgt = (gi + 1) * len(fillers) // n_u_round
                            while fi < tgt:
                                fillers[fi]()
                                fi += 1
                            flush_pend()
                            nf = ((lambda p_=p, r_=rr, a0=av0, a1=av1:
                                   norm_unit(p_, r_, a0, a1)) if last else None)
                            pend = (close, nf)
                            gi += 1
                flush_pend()

                if stage >= 3 and stage < 4:
                    pass
                if stage == 2:
                    # emit remaining projections (no attention to hide them in)
                    for qb in range(1, NQB):
                        for u in proj_units(qb):
                            u()
                if stage >= 4:
                    for half in range(2):
                        for u in outproj_half(NQB - 1, half, last=True):
                            u()

    nc.compile()
    return nc


def _causal_mask_ok(mask):
    m = np.asarray(mask)
    if m.shape != (S, S):
        return False
    return np.array_equal(m.astype(bool), np.triu(np.ones((S, S), bool), k=1))


def _numpy_fallback(x, mask, Wq, bq, Wk, bk, Wv, bv, Wo, bo):
    x = np.asarray(x, np.float64)
    q = (x @ Wq + bq).reshape(B, S, H, DK).transpose(0, 2, 1, 3)
    k = (x @ Wk + bk).reshape(B, S, H, DK).transpose(0, 2, 1, 3)
    v = (x @ Wv + bv).reshape(B, S, H, DK).transpose(0, 2, 1, 3)
    s = np.einsum("bhqd,bhkd->bhqk", q, k) / np.sqrt(DK)
    s = np.where(np.asarray(mask, bool), -np.inf, s)
    s = s - s.max(-1, keepdims=True)
    e = np.exp(s)
    a = e / e.sum(-1, keepdims=True)
    ctx = np.einsum("bhqk,bhkd->bhqd", a, v).transpose(0, 2, 1, 3).reshape(B, S, D)
    return (ctx @ Wo + bo).astype(np.float32)


def _tri_masks():
    m = np.zeros((4, 128, 512), np.float32)
    n = np.arange(512)
    for t in range(4):
        for p_ in range(128):
            m[t, p_, :] = (n >= t * 128 + p_)
    return m.astype(ml_dtypes.bfloat16)


def _make_in_maps(x, Wq, bq, Wk, bk, Wv, bv, Wo):
    Wq, Wk, Wv, Wo = (np.asarray(w, np.float32) for w in (Wq, Wk, Wv, Wo))
    bq, bk, bv = (np.asarray(b_, np.float32) for b_ in (bq, bk, bv))
    masks_np = _tri_masks()
    xts = [np.ascontiguousarray(x[b_].T.astype(ml_dtypes.bfloat16)) for b_ in range(B)]

    in_maps = []
    for c in range(NCORES):
        b_, hs = c // 4, (c % 4) * DL
        in_maps.append({
            "xt": xts[b_],
            "wq": np.ascontiguousarray(Wq[:, hs:hs + DL].astype(ml_dtypes.bfloat16)),
            "wk": np.ascontiguousarray(Wk[:, hs:hs + DL].astype(ml_dtypes.bfloat16)),
            "wv": np.ascontiguousarray(Wv[:, hs:hs + DL].astype(ml_dtypes.bfloat16)),
            "wo": np.ascontiguousarray(Wo[hs:hs + DL, :].astype(ml_dtypes.bfloat16)),
            "bqk": np.ascontiguousarray(np.stack([bq[hs:hs + DL], bk[hs:hs + DL]], 1)),
            "bv": np.ascontiguousarray(bv[hs:hs + DL]),
            "masks": masks_np,
        })
    return in_maps


def kernel(x, mask, Wq, bq, Wk, bk, Wv, bv, Wo, bo):
    x = np.ascontiguousarray(np.asarray(x, np.float32))
    if not _causal_mask_ok(mask):
        return _numpy_fallback(x, mask, Wq, bq, Wk, bk, Wv, bv, Wo, bo)

    from concourse import bass_utils

    if "nc" not in _cache:
        _cache["nc"] = _build(repeat=1)
    nc = _cache["nc"]

    bo = np.asarray(bo, np.float32)
    in_maps = _make_in_maps(x, Wq, bq, Wk, bk, Wv, bv, Wo)

    res = bass_utils.run_bass_kernel_spmd(nc, in_maps, core_ids=list(range(NCORES)))

    out = np.empty((B, S, D), np.float32)
    for b_ in range(B):
        acc = res.results[b_ * 4]["po"].astype(np.float32)
        for g in range(1, 4):
            acc = acc + res.results[b_ * 4 + g]["po"]
        # acc[p, qb, k, s] = outT[k*128+p, qb*512+s]
        out[b_] = acc.transpose(1, 3, 2, 0).reshape(S, D) + bo
    return out
